# revision 11
# baseline (speedup 1.0000x reference)
"""EvolveGCN (3-timestep GraphConv chain) on 8 Trainium2 NeuronCores.

Strategy (graph/data parallel, per sharding hint):
  - Nodes are owned block-cyclically: 128-row block b belongs to core b%8.
    This balances every timestep and keeps ownership consistent across
    timesteps (diff_i = h_i - h_{i-1} is core-local).
  - Edges are bucketed by destination block -> (core, window).  Per window
    (128 destination rows), edges are processed 128 at a time: gather the
    source rows with a PAIR gather (one 512B descriptor fetches table rows
    2k and 2k+1, halving the SWDGE descriptor rate which is the machine
    bottleneck for random-row gathers), build a double-wide weighted
    one-hot O[e, :256] on the vector engine whose halves select the even/
    odd member of the gathered pair, and accumulate
    aggT += G_even^T O_even + G_odd^T O_odd on the tensor engine (PSUM).
    Then h = agg @ W + b per window, plus the per-window epilogue
    (outputs, diff vs h_{i-1}, bf16 shard for the next gather table).
  - The gather table for timestep i+1 (rows = h_i * osc_{i+1}, bf16) is
    assembled on-device with an AllGather collective of the per-core
    shards, plus small DMA injections of the new-node feature rows.
  - Host does index/permutation prep, degree bincounts, edge bucketing and
    output reshuffling; all feature-space FLOPs run on device.
"""

import sys

sys.path.insert(0, "/opt/trn_rl_repo")

import numpy as np
import ml_dtypes

BF16 = ml_dtypes.bfloat16

NCORES = 8
P = 128
CHUNK_TILES = 32  # edge tiles per dma_gather (descs per gather = 128*CT)
ODD_BASE = 512.0  # iota offset selecting the odd half of a gathered pair
PAD_OFF = 99999.0  # one-hot offset that never fires (padding edges)

# Real problem sizes (hardcoded; harness runs exactly this shape).
N_TS = [40000, 45000, 50000]


def _plan_sizes(n_ts):
    B = [(n + P - 1) // P for n in n_ts]  # active 128-row blocks
    W = [(b + NCORES - 1) // NCORES for b in B]  # windows per core
    return B, W


def _pos_of(rows, ts, W, n_ts):
    """Table position of node rows for timestep ts (gather-table layout)."""
    b = rows >> 7
    p = rows & 127
    if ts == 0:
        return ((b % NCORES) * W[0] + b // NCORES) * P + p
    wprev = W[ts - 1]
    nold = NCORES * wprev  # blocks covered by the AllGather region
    pos_old = ((b % NCORES) * wprev + b // NCORES) * P + p
    pos_new = nold * P + (b - nold) * P + p
    return np.where(b < nold, pos_old, pos_new)


def _table_rows(ts, B, W):
    if ts == 0:
        return NCORES * W[0] * P
    nold = NCORES * W[ts - 1]
    return nold * P + max(0, (B[ts] - nold)) * P


def _tail_runs(ts, n_ts, B, W):
    """Contiguous (src_off, dst_off, nrows) runs mapping feat tail rows
    [n_{ts-1}, n_ts) into the ts gather table."""
    rs = np.arange(n_ts[ts - 1], n_ts[ts], dtype=np.int64)
    ds = _pos_of(rs, ts, W, n_ts)
    breaks = np.nonzero(np.diff(ds) != 1)[0]
    starts = np.concatenate([[0], breaks + 1])
    ends = np.concatenate([breaks + 1, [len(rs)]])
    return [(int(s), int(ds[s]), int(e - s)) for s, e in zip(starts, ends)]


def _host_prep(inputs, n_ts):
    """Build all per-core device inputs + shared compile-time schedules."""
    B, W = _plan_sizes(n_ts)
    nts = len(n_ts)
    feats = [np.asarray(inputs[f"feat{i}"], np.float32) for i in range(nts)]
    Ws = [np.asarray(inputs[f"W{i}"], np.float32) for i in range(nts)]
    bs = [np.asarray(inputs[f"b{i}"], np.float32) for i in range(nts)]
    srcs = [np.asarray(inputs[f"src{i}"], np.int64) for i in range(nts)]
    dsts = [np.asarray(inputs[f"dst{i}"], np.int64) for i in range(nts)]

    oscs, iscs = [], []
    for i in range(nts):
        n = n_ts[i]
        outdeg = np.maximum(np.bincount(srcs[i], minlength=n)[:n], 1.0)
        indeg = np.maximum(np.bincount(dsts[i], minlength=n)[:n], 1.0)
        oscs.append((outdeg ** -0.5).astype(np.float32))
        iscs.append((indeg ** -0.5).astype(np.float32))

    sched = {"B": B, "W": W, "R": [_table_rows(i, B, W) for i in range(nts)]}
    per_core = [dict() for _ in range(NCORES)]
    shared = {}

    # ts0 gather table (same for all cores): feat0 * osc0, permuted, bf16.
    tab0 = np.zeros((sched["R"][0], P), dtype=BF16)
    r0 = np.arange(n_ts[0], dtype=np.int64)
    tab0[_pos_of(r0, 0, W, n_ts)] = (feats[0] * oscs[0][:, None]).astype(BF16)
    shared["tab0"] = tab0

    # tail injections for ts1, ts2
    sched["tails"] = {}
    for i in range(1, nts):
        runs = _tail_runs(i, n_ts, B, W)
        rows = feats[i][n_ts[i - 1]: n_ts[i]]
        scale = oscs[i][n_ts[i - 1]: n_ts[i], None]
        shared[f"tail{i}"] = (rows * scale).astype(BF16)
        sched["tails"][i] = runs

    # per-ts edge bucketing (single stream; pair-gather indices)
    sched["TW"], sched["T"], sched["sched_w"] = [], [], []
    for i in range(nts):
        src, dst = srcs[i], dsts[i]
        db = dst >> 7
        owner = db % NCORES
        wloc = db // NCORES
        doff = (dst & 127).astype(np.float32)
        wsc = iscs[i][dst]
        gidx = _pos_of(src, i, W, n_ts).astype(np.int64)
        pair = gidx >> 1
        par = (gidx & 1).astype(np.float32)
        Wi = W[i]

        key_all = owner * Wi + wloc
        cnt = np.bincount(key_all, minlength=NCORES * Wi).reshape(NCORES, Wi)
        TW = np.maximum((-(-cnt // P)).max(axis=0), 1)
        Ti = int(TW.sum())
        sched["TW"].append(TW)
        sched["T"].append(Ti)
        sched["sched_w"].append(np.repeat(np.arange(Wi), TW))

        group_start = np.concatenate([[0], np.cumsum(TW)[:-1]]) * P
        L = Ti * P

        for c in range(NCORES):
            sel = owner == c
            g_w = wloc[sel]
            order = np.argsort(g_w, kind="stable")
            g_w_s = g_w[order]
            gs = np.bincount(g_w_s, minlength=Wi)
            within = np.arange(len(g_w_s)) - np.repeat(
                np.concatenate([[0], np.cumsum(gs)[:-1]]), gs
            )
            slot = group_start[g_w_s] + within

            idx_arr = np.zeros(L, np.int16)
            dof_arr = np.full(L, PAD_OFF, dtype=np.float32)
            wsc_arr = np.zeros(L, dtype=np.float32)
            idx_arr[slot] = pair[sel][order].astype(np.int16)
            dof_arr[slot] = doff[sel][order] + ODD_BASE * par[sel][order]
            wsc_arr[slot] = wsc[sel][order]

            # idx layout: [128, 8*T] int16 (16-partition wrap, replicated)
            per_core[c][f"idx{i}"] = np.tile(
                idx_arr.reshape(8 * Ti, 16).T, (NCORES, 1)
            )
            dw = np.stack([dof_arr, wsc_arr], axis=-1)  # [L, 2]
            per_core[c][f"dw{i}"] = (
                dw.reshape(Ti, P, 2).transpose(1, 0, 2).reshape(P, 2 * Ti)
            )

        # per-core per-window scalar columns
        for c in range(NCORES):
            node = (
                128 * (NCORES * np.arange(Wi)[None, :] + c)
                + np.arange(P)[:, None]
            )  # [128, Wi]
            if i + 1 < nts:
                osc_next = oscs[i + 1]
                valid = node < n_ts[i + 1]
                combo = np.where(
                    valid, osc_next[np.minimum(node, n_ts[i + 1] - 1)], 0.0
                )
                per_core[c][f"combo{i}"] = combo.astype(np.float32)
            if i >= 1:
                per_core[c][f"mask{i}"] = (node < n_ts[i - 1]).astype(
                    np.float32
                )

    # gather chunk schedules (shared): list of (t0, t1) per ts
    sched["chunks"] = []
    for i in range(nts):
        Ti = sched["T"][i]
        ch = [(t0, min(t0 + CHUNK_TILES, Ti)) for t0 in range(0, Ti, CHUNK_TILES)]
        sched["chunks"].append(ch)

    for i in range(nts):
        shared[f"w{i}"] = Ws[i].astype(BF16)
        shared[f"b{i}"] = bs[i].reshape(1, P).astype(BF16)

    in_maps = []
    for c in range(NCORES):
        m = dict(shared)
        m.update(per_core[c])
        in_maps.append(m)
    return in_maps, sched


def _build_program(sched, n_ts):
    import concourse.bacc as bacc
    import concourse.mybir as mybir
    import concourse.tile as tile

    f32 = mybir.dt.float32
    bf16 = mybir.dt.bfloat16
    i16 = mybir.dt.int16
    AOp = mybir.AluOpType
    Act = mybir.ActivationFunctionType

    nts = len(n_ts)
    B, W, R = sched["B"], sched["W"], sched["R"]
    Wmax = max(W)
    Tmax = max(sched["T"])

    nc = bacc.Bacc("TRN2", target_bir_lowering=False, num_swdge_queues=4)

    # ---- I/O declarations ----
    tab0_in = nc.dram_tensor("tab0", [R[0], P], bf16, kind="ExternalInput")
    idx_in, dw_in, w_in, b_in = {}, {}, {}, {}
    combo_in, mask_in, tail_in = {}, {}, {}
    for i in range(nts):
        Ti = sched["T"][i]
        idx_in[i] = nc.dram_tensor(f"idx{i}", [P, 8 * Ti], i16, kind="ExternalInput")
        dw_in[i] = nc.dram_tensor(f"dw{i}", [P, 2 * Ti], f32, kind="ExternalInput")
        w_in[i] = nc.dram_tensor(f"w{i}", [P, P], bf16, kind="ExternalInput")
        b_in[i] = nc.dram_tensor(f"b{i}", [1, P], bf16, kind="ExternalInput")
        if i + 1 < nts:
            combo_in[i] = nc.dram_tensor(
                f"combo{i}", [P, W[i]], f32, kind="ExternalInput"
            )
        if i >= 1:
            mask_in[i] = nc.dram_tensor(f"mask{i}", [P, W[i]], f32, kind="ExternalInput")
            nt = n_ts[i] - n_ts[i - 1]
            tail_in[i] = nc.dram_tensor(f"tail{i}", [nt, P], bf16, kind="ExternalInput")

    houts, douts = {}, {}
    for i in range(nts):
        hpad = W[i + 1] if i + 1 < nts else W[i]  # pad so ts i+1 can read
        houts[i] = nc.dram_tensor(f"h{i}", [hpad * P, P], f32, kind="ExternalOutput")
        if i >= 1:
            douts[i] = nc.dram_tensor(f"d{i}", [W[i] * P, P], f32, kind="ExternalOutput")

    # internal DRAM
    tabs = {0: tab0_in}
    hshard = {}
    for i in range(1, nts):
        tabs[i] = nc.dram_tensor(f"tab{i}", [R[i], P], bf16, addr_space="Shared")
    for i in range(nts - 1):
        hshard[i] = nc.dram_tensor(f"hs{i}", [W[i] * P, P], bf16)

    with tile.TileContext(nc) as tc:
        with (
            tc.tile_pool(name="const", bufs=1) as cp,
            tc.tile_pool(name="idxp", bufs=2) as idxp,
            tc.tile_pool(name="dwp", bufs=2) as dwp,
            tc.tile_pool(name="gbp", bufs=4) as gbp,
            tc.tile_pool(name="op", bufs=16) as op_,
            tc.tile_pool(name="aggp", bufs=2) as aggp,
            tc.tile_pool(name="smallp", bufs=12) as smallp,
            tc.tile_pool(name="psA", bufs=6, space="PSUM") as psA,
            tc.tile_pool(name="psB", bufs=2, space="PSUM") as psB,
        ):
            # ---- constants ----
            iota_i16 = cp.tile([P, 2 * P], i16, tag="iota16", name="iota16")
            nc.gpsimd.iota(iota_i16[:, :P], pattern=[[1, P]], channel_multiplier=0)
            nc.gpsimd.iota(
                iota_i16[:, P:], pattern=[[1, P]], base=int(ODD_BASE),
                channel_multiplier=0,
            )
            iota2 = cp.tile([P, 2 * P], f32, tag="iota2", name="iota2")
            nc.vector.tensor_copy(iota2[:], iota_i16[:])
            ones = cp.tile([1, P], bf16, tag="ones", name="ones")
            nc.vector.memset(ones[:], 1.0)

            wt, bt, combot, maskt = {}, {}, {}, {}
            for i in range(nts):
                wt[i] = cp.tile([P, P], bf16, tag=f"wt{i}", name=f"wt{i}")
                nc.sync.dma_start(wt[i][:], w_in[i][:, :])
                bt[i] = cp.tile([1, P], bf16, tag=f"bt{i}", name=f"bt{i}")
                nc.sync.dma_start(bt[i][:], b_in[i][:, :])
                if i + 1 < nts:
                    combot[i] = cp.tile([P, W[i]], f32, tag=f"combot{i}", name=f"combot{i}")
                    nc.sync.dma_start(combot[i][:], combo_in[i][:, :])
                if i >= 1:
                    maskt[i] = cp.tile([P, W[i]], f32, tag=f"maskt{i}", name=f"maskt{i}")
                    nc.sync.dma_start(maskt[i][:], mask_in[i][:, :])

            for i in range(nts):
                Ti = sched["T"][i]
                Wi = W[i]
                sw = sched["sched_w"][i]

                first_t = {}
                last_t = {}
                for t, w in enumerate(sw):
                    w = int(w)
                    first_t.setdefault(w, t)
                    last_t[w] = t

                idx_sb = idxp.tile([P, 8 * Tmax], i16, tag="idx", name="idx_sb")
                nc.sync.dma_start(idx_sb[:, : 8 * Ti], idx_in[i][:, :])
                dw_sb = dwp.tile([P, 2 * Tmax], f32, tag="dw", name="dw_sb")
                nc.sync.dma_start(dw_sb[:, : 2 * Ti], dw_in[i][:, :])

                agg = aggp.tile([P, Wmax * P], bf16, tag="agg", name="agg")
                # pair view of the gather table: row k = table rows 2k,2k+1
                pview = tabs[i][:, :].rearrange("(a b) c -> a (b c)", b=2)

                pswin = {}
                for gi, (t0, t1) in enumerate(sched["chunks"][i]):
                    ct = t1 - t0
                    gb = gbp.tile([P, CHUNK_TILES, 2 * P], bf16, tag="gb", name="gb")
                    nc.gpsimd.dma_gather(
                        out_ap=gb[:, :ct, :],
                        in_ap=pview,
                        idxs_ap=idx_sb[:, 8 * t0 : 8 * t1],
                        num_idxs=P * ct,
                        num_idxs_reg=P * ct,
                        elem_size=2 * P,
                        single_packet=False,
                        queue_num=gi % 4,
                    )
                    for t in range(t0, t1):
                        w = int(sw[t])
                        o_t = op_.tile([P, 2 * P], bf16, tag="onehot", name="o_t")
                        nc.vector.tensor_scalar(
                            out=o_t[:],
                            in0=iota2[:],
                            scalar1=dw_sb[:, 2 * t : 2 * t + 1],
                            scalar2=dw_sb[:, 2 * t + 1 : 2 * t + 2],
                            op0=AOp.is_equal,
                            op1=AOp.mult,
                        )
                        if t == first_t[w]:
                            pswin[w] = psA.tile([P, P], f32, tag="scatps", name="scatps")
                        nc.tensor.matmul(
                            out=pswin[w][:],
                            lhsT=gb[:, t - t0, 0:P],
                            rhs=o_t[:, 0:P],
                            start=(t == first_t[w]),
                            stop=False,
                        )
                        nc.tensor.matmul(
                            out=pswin[w][:],
                            lhsT=gb[:, t - t0, P : 2 * P],
                            rhs=o_t[:, P : 2 * P],
                            start=False,
                            stop=(t == last_t[w]),
                        )
                        if t == last_t[w]:
                            nc.scalar.activation(
                                agg[:, w * P : (w + 1) * P], pswin[w][:],
                                Act.Copy,
                            )
                            del pswin[w]

                # ---- per-window h / outputs ----
                for w in range(Wi):
                    hps = psB.tile([P, P], f32, tag="hps", name="hps")
                    nc.tensor.matmul(
                        out=hps[:], lhsT=ones[:], rhs=bt[i][:],
                        start=True, stop=False,
                    )
                    nc.tensor.matmul(
                        out=hps[:],
                        lhsT=agg[:, w * P : (w + 1) * P],
                        rhs=wt[i][:],
                        start=False,
                        stop=True,
                    )
                    h_sb = smallp.tile([P, P], f32, tag="hsb", name="h_sb")
                    nc.scalar.activation(h_sb[:], hps[:], Act.Copy)
                    nc.sync.dma_start(houts[i][w * P : (w + 1) * P, :], h_sb[:])
                    if i + 1 < nts:
                        sh = smallp.tile([P, P], bf16, tag="shsb", name="sh")
                        nc.scalar.activation(
                            sh[:], hps[:], Act.Copy, scale=combot[i][:, w : w + 1]
                        )
                        nc.sync.dma_start(hshard[i][w * P : (w + 1) * P, :], sh[:])
                    if i >= 1:
                        hp = smallp.tile([P, P], f32, tag="hprev", name="hp")
                        nc.sync.dma_start(hp[:], houts[i - 1][w * P : (w + 1) * P, :])
                        hpm = smallp.tile([P, P], f32, tag="hprevm", name="hpm")
                        nc.scalar.activation(
                            hpm[:], hp[:], Act.Copy, scale=maskt[i][:, w : w + 1]
                        )
                        d_sb = smallp.tile([P, P], f32, tag="dsb", name="d_sb")
                        nc.vector.tensor_tensor(
                            out=d_sb[:], in0=h_sb[:], in1=hpm[:], op=AOp.subtract
                        )
                        nc.sync.dma_start(douts[i][w * P : (w + 1) * P, :], d_sb[:])

                # ---- table assembly for next ts ----
                if i + 1 < nts:
                    nc.gpsimd.collective_compute(
                        "AllGather",
                        AOp.bypass,
                        replica_groups=[list(range(NCORES))],
                        ins=[hshard[i][:, :].opt()],
                        outs=[tabs[i + 1][0 : NCORES * Wi * P, :].opt()],
                    )
                    for (so, do, nr) in sched["tails"][i + 1]:
                        nc.sync.dma_start(
                            tabs[i + 1][do : do + nr, :],
                            tail_in[i + 1][so : so + nr, :],
                        )

    nc.compile()
    return nc


def _assemble(results, sched, n_ts):
    """Reassemble per-core block-cyclic window outputs into global arrays."""
    W = sched["W"]
    nts = len(n_ts)
    feats_out, diffs_out = [], []
    for i in range(nts):
        Wi = W[i]
        h = np.stack(
            [results[c][f"h{i}"][: Wi * P].reshape(Wi, P, P) for c in range(NCORES)],
            axis=1,
        )  # [Wi, 8, 128, 128]
        feats_out.append(h.reshape(Wi * NCORES * P, P)[: n_ts[i]].copy())
        if i == 0:
            diffs_out.append(feats_out[0].copy())
        else:
            d = np.stack(
                [results[c][f"d{i}"].reshape(Wi, P, P) for c in range(NCORES)],
                axis=1,
            )
            diffs_out.append(d.reshape(Wi * NCORES * P, P)[: n_ts[i]].copy())
    return tuple(feats_out) + tuple(diffs_out)


_PROGRAM_CACHE = {}

# test.py sets PROFILE=True to capture neuron-profile results into LAST_RESULT.
PROFILE = False
LAST_RESULT = None


def kernel(**inputs):
    global LAST_RESULT
    from concourse.bass_utils import run_bass_kernel_spmd

    n_ts = N_TS
    in_maps, sched = _host_prep(inputs, n_ts)
    key = tuple(sched["T"])
    if key not in _PROGRAM_CACHE:
        _PROGRAM_CACHE[key] = _build_program(sched, n_ts)
    nc = _PROGRAM_CACHE[key]
    kw = {"trace": True} if PROFILE else {}
    res = run_bass_kernel_spmd(nc, in_maps, list(range(NCORES)), **kw)
    LAST_RESULT = res
    return _assemble(res.results, sched, n_ts)


# revision 13
# speedup vs baseline: 1.9111x; 1.9111x over previous
"""EvolveGCN (3-timestep GraphConv chain) on 8 Trainium2 NeuronCores.

Strategy (graph/data parallel, per sharding hint):
  - Nodes are owned block-cyclically: 128-row block b belongs to core b%8.
    This balances every timestep and keeps ownership consistent across
    timesteps (diff_i = h_i - h_{i-1} is core-local).
  - Edges are bucketed by destination block -> (core, window).  Per window
    (128 destination rows), edges are processed 128 at a time: gather the
    source rows with a PAIR gather (one 512B descriptor fetches table rows
    2k and 2k+1, halving the SWDGE descriptor rate which is the machine
    bottleneck for random-row gathers), build a double-wide weighted
    one-hot O[e, :256] on the vector engine whose halves select the even/
    odd member of the gathered pair, and accumulate
    aggT += G_even^T O_even + G_odd^T O_odd on the tensor engine (PSUM).
    Then h = agg @ W + b per window, plus the per-window epilogue
    (outputs, diff vs h_{i-1}, bf16 shard for the next gather table).
  - The gather table for timestep i+1 (rows = h_i * osc_{i+1}, bf16) is
    assembled on-device with an AllGather collective of the per-core
    shards, plus small DMA injections of the new-node feature rows.
  - Host does index/permutation prep, degree bincounts, edge bucketing and
    output reshuffling; all feature-space FLOPs run on device.
"""

import sys

sys.path.insert(0, "/opt/trn_rl_repo")

import numpy as np
import ml_dtypes

BF16 = ml_dtypes.bfloat16

NCORES = 8
P = 128
CHUNK_TILES = 32  # edge tiles per dma_gather (descs per gather = 128*CT)
ODD_BASE = 512.0  # iota offset selecting the odd half of a gathered pair
PAD_OFF = 99999.0  # one-hot offset that never fires (padding edges)

# Real problem sizes (hardcoded; harness runs exactly this shape).
N_TS = [40000, 45000, 50000]


def _plan_sizes(n_ts):
    B = [(n + P - 1) // P for n in n_ts]  # active 128-row blocks
    W = [(b + NCORES - 1) // NCORES for b in B]  # windows per core
    return B, W


def _pos_of(rows, ts, W, n_ts):
    """Table position of node rows for timestep ts (gather-table layout)."""
    b = rows >> 7
    p = rows & 127
    if ts == 0:
        return ((b % NCORES) * W[0] + b // NCORES) * P + p
    wprev = W[ts - 1]
    nold = NCORES * wprev  # blocks covered by the AllGather region
    pos_old = ((b % NCORES) * wprev + b // NCORES) * P + p
    pos_new = nold * P + (b - nold) * P + p
    return np.where(b < nold, pos_old, pos_new)


def _table_rows(ts, B, W):
    if ts == 0:
        return NCORES * W[0] * P
    nold = NCORES * W[ts - 1]
    return nold * P + max(0, (B[ts] - nold)) * P


def _tail_runs(ts, n_ts, B, W):
    """Contiguous (src_off, dst_off, nrows) runs mapping feat tail rows
    [n_{ts-1}, n_ts) into the ts gather table."""
    rs = np.arange(n_ts[ts - 1], n_ts[ts], dtype=np.int64)
    ds = _pos_of(rs, ts, W, n_ts)
    breaks = np.nonzero(np.diff(ds) != 1)[0]
    starts = np.concatenate([[0], breaks + 1])
    ends = np.concatenate([breaks + 1, [len(rs)]])
    return [(int(s), int(ds[s]), int(e - s)) for s, e in zip(starts, ends)]


def _host_prep(inputs, n_ts):
    """Build all per-core device inputs + shared compile-time schedules."""
    B, W = _plan_sizes(n_ts)
    nts = len(n_ts)
    feats = [np.asarray(inputs[f"feat{i}"], np.float32) for i in range(nts)]
    Ws = [np.asarray(inputs[f"W{i}"], np.float32) for i in range(nts)]
    bs = [np.asarray(inputs[f"b{i}"], np.float32) for i in range(nts)]
    srcs = [np.asarray(inputs[f"src{i}"], np.int64) for i in range(nts)]
    dsts = [np.asarray(inputs[f"dst{i}"], np.int64) for i in range(nts)]

    oscs, iscs = [], []
    for i in range(nts):
        n = n_ts[i]
        outdeg = np.maximum(np.bincount(srcs[i], minlength=n)[:n], 1.0)
        indeg = np.maximum(np.bincount(dsts[i], minlength=n)[:n], 1.0)
        oscs.append((outdeg ** -0.5).astype(np.float32))
        iscs.append((indeg ** -0.5).astype(np.float32))

    sched = {"B": B, "W": W, "R": [_table_rows(i, B, W) for i in range(nts)]}
    per_core = [dict() for _ in range(NCORES)]
    shared = {}

    # ts0 gather table (same for all cores): feat0 * osc0, permuted, bf16.
    tab0 = np.zeros((sched["R"][0], P), dtype=BF16)
    r0 = np.arange(n_ts[0], dtype=np.int64)
    tab0[_pos_of(r0, 0, W, n_ts)] = (feats[0] * oscs[0][:, None]).astype(BF16)
    shared["tab0"] = tab0

    # tail injections for ts1, ts2
    sched["tails"] = {}
    for i in range(1, nts):
        runs = _tail_runs(i, n_ts, B, W)
        rows = feats[i][n_ts[i - 1]: n_ts[i]]
        scale = oscs[i][n_ts[i - 1]: n_ts[i], None]
        shared[f"tail{i}"] = (rows * scale).astype(BF16)
        sched["tails"][i] = runs

    # per-ts edge bucketing (single stream; pair-gather indices)
    sched["TW"], sched["T"], sched["sched_w"] = [], [], []
    for i in range(nts):
        src, dst = srcs[i], dsts[i]
        db = dst >> 7
        owner = db % NCORES
        wloc = db // NCORES
        doff = (dst & 127).astype(np.float32)
        wsc = iscs[i][dst]
        gidx = _pos_of(src, i, W, n_ts).astype(np.int64)
        pair = gidx >> 1
        par = (gidx & 1).astype(np.float32)
        Wi = W[i]

        key_all = owner * Wi + wloc
        cnt = np.bincount(key_all, minlength=NCORES * Wi).reshape(NCORES, Wi)
        TW = np.maximum((-(-cnt // P)).max(axis=0), 1)
        Ti = int(TW.sum())
        sched["TW"].append(TW)
        sched["T"].append(Ti)
        sched["sched_w"].append(np.repeat(np.arange(Wi), TW))

        group_start = np.concatenate([[0], np.cumsum(TW)[:-1]]) * P
        L = Ti * P

        for c in range(NCORES):
            sel = owner == c
            g_w = wloc[sel]
            order = np.argsort(g_w, kind="stable")
            g_w_s = g_w[order]
            gs = np.bincount(g_w_s, minlength=Wi)
            within = np.arange(len(g_w_s)) - np.repeat(
                np.concatenate([[0], np.cumsum(gs)[:-1]]), gs
            )
            slot = group_start[g_w_s] + within

            idx_arr = np.zeros(L, np.int16)
            dof_arr = np.full(L, PAD_OFF, dtype=np.float32)
            wsc_arr = np.zeros(L, dtype=np.float32)
            idx_arr[slot] = pair[sel][order].astype(np.int16)
            dof_arr[slot] = doff[sel][order] + ODD_BASE * par[sel][order]
            wsc_arr[slot] = wsc[sel][order]

            # idx layout: [128, 8*T] int16 (16-partition wrap, replicated)
            per_core[c][f"idx{i}"] = np.tile(
                idx_arr.reshape(8 * Ti, 16).T, (NCORES, 1)
            )
            dw = np.stack([dof_arr, wsc_arr], axis=-1)  # [L, 2]
            per_core[c][f"dw{i}"] = (
                dw.reshape(Ti, P, 2).transpose(1, 0, 2).reshape(P, 2 * Ti)
            )

        # per-core per-window scalar columns
        for c in range(NCORES):
            node = (
                128 * (NCORES * np.arange(Wi)[None, :] + c)
                + np.arange(P)[:, None]
            )  # [128, Wi]
            if i + 1 < nts:
                osc_next = oscs[i + 1]
                valid = node < n_ts[i + 1]
                combo = np.where(
                    valid, osc_next[np.minimum(node, n_ts[i + 1] - 1)], 0.0
                )
                per_core[c][f"combo{i}"] = combo.astype(np.float32)
            if i >= 1:
                per_core[c][f"mask{i}"] = (node < n_ts[i - 1]).astype(
                    np.float32
                )

    # gather chunk schedules (shared): list of (t0, t1) per ts
    sched["chunks"] = []
    for i in range(nts):
        Ti = sched["T"][i]
        ch = [(t0, min(t0 + CHUNK_TILES, Ti)) for t0 in range(0, Ti, CHUNK_TILES)]
        sched["chunks"].append(ch)

    for i in range(nts):
        shared[f"w{i}"] = Ws[i].astype(BF16)
        shared[f"b{i}"] = bs[i].reshape(1, P).astype(BF16)

    in_maps = []
    for c in range(NCORES):
        m = dict(shared)
        m.update(per_core[c])
        in_maps.append(m)
    return in_maps, sched


def _build_program(sched, n_ts):
    import concourse.bacc as bacc
    import concourse.mybir as mybir
    import concourse.tile as tile

    f32 = mybir.dt.float32
    bf16 = mybir.dt.bfloat16
    i16 = mybir.dt.int16
    AOp = mybir.AluOpType
    Act = mybir.ActivationFunctionType

    nts = len(n_ts)
    B, W, R = sched["B"], sched["W"], sched["R"]
    Wmax = max(W)
    Tmax = max(sched["T"])

    nc = bacc.Bacc("TRN2", target_bir_lowering=False, num_swdge_queues=4)

    # ---- I/O declarations ----
    tab0_in = nc.dram_tensor("tab0", [R[0], P], bf16, kind="ExternalInput")
    idx_in, dw_in, w_in, b_in = {}, {}, {}, {}
    combo_in, mask_in, tail_in = {}, {}, {}
    for i in range(nts):
        Ti = sched["T"][i]
        idx_in[i] = nc.dram_tensor(f"idx{i}", [P, 8 * Ti], i16, kind="ExternalInput")
        dw_in[i] = nc.dram_tensor(f"dw{i}", [P, 2 * Ti], f32, kind="ExternalInput")
        w_in[i] = nc.dram_tensor(f"w{i}", [P, P], bf16, kind="ExternalInput")
        b_in[i] = nc.dram_tensor(f"b{i}", [1, P], bf16, kind="ExternalInput")
        if i + 1 < nts:
            combo_in[i] = nc.dram_tensor(
                f"combo{i}", [P, W[i]], f32, kind="ExternalInput"
            )
        if i >= 1:
            mask_in[i] = nc.dram_tensor(f"mask{i}", [P, W[i]], f32, kind="ExternalInput")
            nt = n_ts[i] - n_ts[i - 1]
            tail_in[i] = nc.dram_tensor(f"tail{i}", [nt, P], bf16, kind="ExternalInput")

    houts, douts = {}, {}
    for i in range(nts):
        hpad = W[i + 1] if i + 1 < nts else W[i]  # pad so ts i+1 can read
        houts[i] = nc.dram_tensor(f"h{i}", [hpad * P, P], f32, kind="ExternalOutput")
        if i >= 1:
            douts[i] = nc.dram_tensor(f"d{i}", [W[i] * P, P], f32, kind="ExternalOutput")

    # internal DRAM
    tabs = {0: tab0_in}
    hshard = {}
    for i in range(1, nts):
        tabs[i] = nc.dram_tensor(f"tab{i}", [R[i], P], bf16, addr_space="Shared")
    for i in range(nts - 1):
        hshard[i] = nc.dram_tensor(f"hs{i}", [W[i] * P, P], bf16)

    with tile.TileContext(nc) as tc:
        with (
            tc.tile_pool(name="const", bufs=1) as cp,
            tc.tile_pool(name="idxp", bufs=2) as idxp,
            tc.tile_pool(name="dwp", bufs=2) as dwp,
            tc.tile_pool(name="gbp", bufs=4) as gbp,
            tc.tile_pool(name="op", bufs=16) as op_,
            tc.tile_pool(name="aggp", bufs=2) as aggp,
            tc.tile_pool(name="smallp", bufs=12) as smallp,
            tc.tile_pool(name="psA", bufs=6, space="PSUM") as psA,
            tc.tile_pool(name="psB", bufs=2, space="PSUM") as psB,
        ):
            # ---- constants ----
            iota_i16 = cp.tile([P, 2 * P], i16, tag="iota16", name="iota16")
            nc.gpsimd.iota(iota_i16[:, :P], pattern=[[1, P]], channel_multiplier=0)
            nc.gpsimd.iota(
                iota_i16[:, P:], pattern=[[1, P]], base=int(ODD_BASE),
                channel_multiplier=0,
            )
            iota2 = cp.tile([P, 2 * P], f32, tag="iota2", name="iota2")
            nc.vector.tensor_copy(iota2[:], iota_i16[:])
            ones = cp.tile([1, P], bf16, tag="ones", name="ones")
            nc.vector.memset(ones[:], 1.0)
            dummy_o = cp.tile([P, 2 * P], bf16, tag="dummy_o", name="dummy_o")
            nc.vector.tensor_copy(dummy_o[:], iota_i16[:])

            wt, bt, combot, maskt = {}, {}, {}, {}
            for i in range(nts):
                wt[i] = cp.tile([P, P], bf16, tag=f"wt{i}", name=f"wt{i}")
                nc.sync.dma_start(wt[i][:], w_in[i][:, :])
                bt[i] = cp.tile([1, P], bf16, tag=f"bt{i}", name=f"bt{i}")
                nc.sync.dma_start(bt[i][:], b_in[i][:, :])
                if i + 1 < nts:
                    combot[i] = cp.tile([P, W[i]], f32, tag=f"combot{i}", name=f"combot{i}")
                    nc.sync.dma_start(combot[i][:], combo_in[i][:, :])
                if i >= 1:
                    maskt[i] = cp.tile([P, W[i]], f32, tag=f"maskt{i}", name=f"maskt{i}")
                    nc.sync.dma_start(maskt[i][:], mask_in[i][:, :])

            for i in range(nts):
                Ti = sched["T"][i]
                Wi = W[i]
                sw = sched["sched_w"][i]

                first_t = {}
                last_t = {}
                for t, w in enumerate(sw):
                    w = int(w)
                    first_t.setdefault(w, t)
                    last_t[w] = t

                idx_sb = idxp.tile([P, 8 * Tmax], i16, tag="idx", name="idx_sb")
                nc.sync.dma_start(idx_sb[:, : 8 * Ti], idx_in[i][:, :])
                dw_sb = dwp.tile([P, 2 * Tmax], f32, tag="dw", name="dw_sb")
                nc.sync.dma_start(dw_sb[:, : 2 * Ti], dw_in[i][:, :])

                agg = aggp.tile([P, Wmax * P], bf16, tag="agg", name="agg")
                # pair view of the gather table: row k = table rows 2k,2k+1
                pview = tabs[i][:, :].rearrange("(a b) c -> a (b c)", b=2)

                pswin = {}
                for gi, (t0, t1) in enumerate(sched["chunks"][i]):
                    ct = t1 - t0
                    gb = gbp.tile([P, CHUNK_TILES, 2 * P], bf16, tag="gb", name="gb")
                    nc.gpsimd.dma_gather(
                        out_ap=gb[:, :ct, :],
                        in_ap=pview,
                        idxs_ap=idx_sb[:, 8 * t0 : 8 * t1],
                        num_idxs=P * ct,
                        num_idxs_reg=P * ct,
                        elem_size=2 * P,
                        single_packet=False,
                        queue_num=gi % 4,
                    )
                    for t in range(t0, t1):
                        w = int(sw[t])
                        import os as _os
                        if _os.environ.get("NO_OHBUILD"):
                            o_t = {"__const": True}
                        else:
                            o_t = op_.tile([P, 2 * P], bf16, tag="onehot", name="o_t")
                            nc.vector.tensor_scalar(
                                out=o_t[:],
                                in0=iota2[:],
                                scalar1=dw_sb[:, 2 * t : 2 * t + 1],
                                scalar2=dw_sb[:, 2 * t + 1 : 2 * t + 2],
                                op0=AOp.is_equal,
                                op1=AOp.mult,
                            )
                        if isinstance(o_t, dict):
                            class _W:  # fake AP provider using const iota tile
                                pass
                            o_t = None
                        if t == first_t[w]:
                            pswin[w] = psA.tile([P, P], f32, tag="scatps", name="scatps")
                        _rhs = dummy_o if o_t is None else o_t
                        nc.tensor.matmul(
                            out=pswin[w][:],
                            lhsT=gb[:, t - t0, 0:P],
                            rhs=(_rhs[:, 0:P] if o_t is None else o_t[:, 0:P]),
                            start=(t == first_t[w]),
                            stop=False,
                        )
                        nc.tensor.matmul(
                            out=pswin[w][:],
                            lhsT=gb[:, t - t0, P : 2 * P],
                            rhs=(_rhs[:, P : 2 * P] if o_t is None else o_t[:, P : 2 * P]),
                            start=False,
                            stop=(t == last_t[w]),
                        )
                        if t == last_t[w]:
                            nc.scalar.activation(
                                agg[:, w * P : (w + 1) * P], pswin[w][:],
                                Act.Copy,
                            )
                            del pswin[w]

                # ---- per-window h / outputs ----
                for w in range(Wi):
                    hps = psB.tile([P, P], f32, tag="hps", name="hps")
                    nc.tensor.matmul(
                        out=hps[:], lhsT=ones[:], rhs=bt[i][:],
                        start=True, stop=False,
                    )
                    nc.tensor.matmul(
                        out=hps[:],
                        lhsT=agg[:, w * P : (w + 1) * P],
                        rhs=wt[i][:],
                        start=False,
                        stop=True,
                    )
                    h_sb = smallp.tile([P, P], f32, tag="hsb", name="h_sb")
                    nc.scalar.activation(h_sb[:], hps[:], Act.Copy)
                    nc.sync.dma_start(houts[i][w * P : (w + 1) * P, :], h_sb[:])
                    if i + 1 < nts:
                        sh = smallp.tile([P, P], bf16, tag="shsb", name="sh")
                        nc.scalar.activation(
                            sh[:], hps[:], Act.Copy, scale=combot[i][:, w : w + 1]
                        )
                        nc.sync.dma_start(hshard[i][w * P : (w + 1) * P, :], sh[:])
                    if i >= 1:
                        hp = smallp.tile([P, P], f32, tag="hprev", name="hp")
                        nc.sync.dma_start(hp[:], houts[i - 1][w * P : (w + 1) * P, :])
                        hpm = smallp.tile([P, P], f32, tag="hprevm", name="hpm")
                        nc.scalar.activation(
                            hpm[:], hp[:], Act.Copy, scale=maskt[i][:, w : w + 1]
                        )
                        d_sb = smallp.tile([P, P], f32, tag="dsb", name="d_sb")
                        nc.vector.tensor_tensor(
                            out=d_sb[:], in0=h_sb[:], in1=hpm[:], op=AOp.subtract
                        )
                        nc.sync.dma_start(douts[i][w * P : (w + 1) * P, :], d_sb[:])

                # ---- table assembly for next ts ----
                if i + 1 < nts:
                    nc.gpsimd.collective_compute(
                        "AllGather",
                        AOp.bypass,
                        replica_groups=[list(range(NCORES))],
                        ins=[hshard[i][:, :].opt()],
                        outs=[tabs[i + 1][0 : NCORES * Wi * P, :].opt()],
                    )
                    for (so, do, nr) in sched["tails"][i + 1]:
                        nc.sync.dma_start(
                            tabs[i + 1][do : do + nr, :],
                            tail_in[i + 1][so : so + nr, :],
                        )

    nc.compile()
    return nc


def _assemble(results, sched, n_ts):
    """Reassemble per-core block-cyclic window outputs into global arrays."""
    W = sched["W"]
    nts = len(n_ts)
    feats_out, diffs_out = [], []
    for i in range(nts):
        Wi = W[i]
        h = np.stack(
            [results[c][f"h{i}"][: Wi * P].reshape(Wi, P, P) for c in range(NCORES)],
            axis=1,
        )  # [Wi, 8, 128, 128]
        feats_out.append(h.reshape(Wi * NCORES * P, P)[: n_ts[i]].copy())
        if i == 0:
            diffs_out.append(feats_out[0].copy())
        else:
            d = np.stack(
                [results[c][f"d{i}"].reshape(Wi, P, P) for c in range(NCORES)],
                axis=1,
            )
            diffs_out.append(d.reshape(Wi * NCORES * P, P)[: n_ts[i]].copy())
    return tuple(feats_out) + tuple(diffs_out)


_PROGRAM_CACHE = {}

# test.py sets PROFILE=True to capture neuron-profile results into LAST_RESULT.
PROFILE = False
LAST_RESULT = None


def kernel(**inputs):
    global LAST_RESULT
    from concourse.bass_utils import run_bass_kernel_spmd

    n_ts = N_TS
    in_maps, sched = _host_prep(inputs, n_ts)
    key = tuple(sched["T"])
    if key not in _PROGRAM_CACHE:
        _PROGRAM_CACHE[key] = _build_program(sched, n_ts)
    nc = _PROGRAM_CACHE[key]
    kw = {"trace": True} if PROFILE else {}
    res = run_bass_kernel_spmd(nc, in_maps, list(range(NCORES)), **kw)
    LAST_RESULT = res
    return _assemble(res.results, sched, n_ts)
